# revision 10
# baseline (speedup 1.0000x reference)
"""GCN (5-layer) + global mean pool + MLP head on 8 trn2 NeuronCores.

Strategy (v2, feature-transposed + ap_gather):
  - Factorized GCN norm: y = dis * (h @ W); h'[v] = relu(dis[v]*(sum_in y + y[v]) + b).
  - Layout: features on partitions. Core c owns nodes [c*12500,(c+1)*12500),
    split into 8 groups of <=1568 dests; SBUF state tiles are [128, 1568]
    where partition 16g+f holds feature f of group g's dests.
  - Per layer, y is exchanged via AllGather into a LOCAL dram tensor
    ag8 [128, 12544] (row 16c+f = feature f of core c, col = canonical
    node column). Window w (= core w's slice) is DMA-replicated into SBUF
    [128, 12546] (all 8 groups see all of core w's nodes; 2 zero cols).
  - Message gathering runs on GPSIMD via ap_gather: each of the 8 Q7 cores
    gathers its group's in-edges' source columns. Columns are slot-major
    (k-major runs of constant K, dests degree-sorted per window); DVE
    tree-reduces runs and accumulates into acc; a realign ap_gather maps
    window-rank order back to canonical order.
  - Epilogue: h = relu(dis*acc + b); y' = dis * (kron(I8,W_next)^T @ h) via a
    single 128x128 PE matmul (block-diagonal weights keep group structure).
  - Pooling: partial per-core graph sums from own h5 (ap_gather + reduce),
    cross-group sum via a one-hot matmul, tiny AllGather of partials, then
    per-core assembly gather + MLP head.
"""
import numpy as np

import concourse.bass as bass
import concourse.bacc as bacc
import concourse.tile as tile
import concourse.mybir as mybir
from concourse.bass2jax import run_bass_via_pjrt

F32 = mybir.dt.float32
I16 = mybir.dt.int16
AL = mybir.AluOpType

N_NODES = 100000
N_EDGES = 3200000
N_GRAPHS = 1000
HID = 16
C = 8                    # cores
P = 128
NPC = N_NODES // C       # 12500
G8 = 8                   # partition groups (= gpsimd cores)
GSZ = 1568               # padded dests per group (8*1568 = 12544)
NPAD = G8 * GSZ          # 12544
WIN_ELEMS = NPAD + 2     # window cols incl 2 zero cols
ZCOL = NPAD              # zero column index in window
GPC = N_GRAPHS // C      # 125 graphs per core
PG = 128                 # pool slots per core (127 graphs max + zero slot)
PZSLOT = PG - 1          # zero slot in pool partials
HSRC = GSZ + 2           # h tile cols incl zero cols
CHUNK = 5120             # target idxs per ap_gather instruction
RRELU_SLOPE = (1.0 / 8.0 + 1.0 / 3.0) / 2.0


def _wrap_groups(idx_per_group):
    """[G8, n] int array -> [128, n//16] wrapped int16 (idx j of group g at
    partition 16g + j%16, col j//16)."""
    g8, n = idx_per_group.shape
    assert g8 == G8 and n % 16 == 0
    out = np.empty((P, n // 16), dtype=np.int16)
    for g in range(G8):
        out[16 * g:16 * g + 16, :] = idx_per_group[g].reshape(-1, 16).T
    return out


def _build_runs(Ks):
    """Monotone non-increasing K per rank -> list of (i0, L, K) runs, K>=1."""
    runs = []
    i = 0
    n = len(Ks)
    while i < n and Ks[i] > 0:
        j = i
        while j < n and Ks[j] == Ks[i]:
            j += 1
        runs.append((i, j - i, int(Ks[i])))
        i = j
    return runs, i  # i = first zero-K rank


def _chunk_runs(runs, chunk_max):
    """Pack runs into chunks of <= chunk_max columns (padded to %32).
    Returns list of (n_idxs_padded, [(i0, L, K, base_col)])."""
    chunks = []
    cur = []
    cur_cols = 0
    for (i0, L, K) in runs:
        cols = L * K
        # oversize run: split by rank range
        while cols > chunk_max:
            take_L = max(1, chunk_max // K)
            if cur_cols + take_L * K > chunk_max and cur:
                chunks.append((cur_cols, cur))
                cur, cur_cols = [], 0
            take_L = min(take_L, L)
            cur.append((i0, take_L, K, cur_cols))
            cur_cols += take_L * K
            i0 += take_L
            L -= take_L
            cols = L * K
            if cur_cols >= chunk_max - 32:
                chunks.append((cur_cols, cur))
                cur, cur_cols = [], 0
        if cols == 0:
            continue
        if cur_cols + cols > chunk_max and cur:
            chunks.append((cur_cols, cur))
            cur, cur_cols = [], 0
        cur.append((i0, L, K, cur_cols))
        cur_cols += cols
    if cur:
        chunks.append((cur_cols, cur))
    out = []
    for (n, rr) in chunks:
        npad = (n + 31) // 32 * 32
        out.append((npad, rr))
    return out


def _preprocess(x, edge_index, batch):
    x = np.asarray(x, dtype=np.float32)
    src = np.asarray(edge_index[0], dtype=np.int64)
    dst = np.asarray(edge_index[1], dtype=np.int64)
    batch = np.asarray(batch, dtype=np.int64)

    deg = np.bincount(dst, minlength=N_NODES).astype(np.float32) + 1.0
    dis = 1.0 / np.sqrt(deg)

    # --- canonical positions: per core, sort by total degree desc, round-robin
    group_of = np.empty(N_NODES, dtype=np.int64)
    rank_of = np.empty(N_NODES, dtype=np.int64)
    for c in range(C):
        lo = c * NPC
        d = deg[lo:lo + NPC]
        order = np.argsort(-d, kind="stable")
        pos = np.empty(NPC, dtype=np.int64)
        pos[order] = np.arange(NPC)
        group_of[lo:lo + NPC] = pos % G8
        rank_of[lo:lo + NPC] = pos // G8
    col_of = group_of * GSZ + rank_of      # core-local canonical column
    wcol_of = col_of                       # window-local column (same)

    # --- per (dest, window) in-edge counts
    wsrc = src // NPC
    key_dw = dst * C + wsrc
    cnt = np.bincount(key_dw, minlength=N_NODES * C).reshape(N_NODES, C)

    # --- per (core, window, group): window ordering by count desc
    # wrank[d, w] = rank of dest d within (its core, its group) for window w
    wrank = np.empty((N_NODES, C), dtype=np.int64)
    Ks = np.zeros((C, GSZ), dtype=np.int64)   # [window, rank] cross core+group max
    for c in range(C):
        lo = c * NPC
        for g in range(G8):
            sel = np.flatnonzero(group_of[lo:lo + NPC] == g) + lo
            for w in range(C):
                cw = cnt[sel, w]
                order = np.argsort(-cw, kind="stable")
                rk = np.empty(len(sel), dtype=np.int64)
                rk[order] = np.arange(len(sel))
                wrank[sel, w] = rk
                Ks[w, :len(sel)] = np.maximum(Ks[w, :len(sel)], cw[order])

    # --- runs and chunks per window (shared across cores/groups)
    win_chunks = []   # per window: list of (n_idxs, [(i0,L,K,base)])
    win_zero_from = []
    for w in range(C):
        runs, zero_from = _build_runs(Ks[w])
        win_chunks.append(_chunk_runs(runs, CHUNK))
        win_zero_from.append(zero_from)

    # --- edge -> (sbuf col, partition) token assembly
    # chunk col offsets per window (sbuf idx tile layout: concat windows/chunks)
    chunk_base = []   # [w][ci] -> base idx-col (in idxs, not wrapped)
    total_idx = 0
    for w in range(C):
        bases = []
        for (n, _rr) in win_chunks[w]:
            bases.append(total_idx)
            total_idx += n
        chunk_base.append(bases)
    GCOLS = total_idx // 16

    # per-edge slot within (dest, window): stable order
    eorder = np.lexsort((src, wsrc, dst))
    sd, sw = dst[eorder], wsrc[eorder]
    gkey = sd * C + sw
    starts = np.concatenate([[True], gkey[1:] != gkey[:-1]])
    first = np.flatnonzero(starts)
    gidx = np.cumsum(starts) - 1
    slot = np.arange(len(eorder)) - first[gidx]

    # map (window, rank_in_window, slot) -> within-chunk column
    # build per window lookup arrays: for rank i: run id; run (i0, L, K, base, chunk_ci)
    # col_in_chunk = base + s*L + (i - i0); idx col global = chunk_base[w][ci] + col
    e_rank = wrank[sd, sw]
    e_group = group_of[sd]
    e_core = sd // NPC
    e_val = wcol_of[src[eorder]]

    idx_tok = [np.full((G8, total_idx), ZCOL, dtype=np.int64) for _ in range(C)]
    for w in range(C):
        # rank -> (L, K, i0, gbase) arrays
        rmap_L = np.zeros(GSZ, dtype=np.int64)
        rmap_K = np.zeros(GSZ, dtype=np.int64)
        rmap_i0 = np.zeros(GSZ, dtype=np.int64)
        rmap_gb = np.full(GSZ, -1, dtype=np.int64)
        for ci, (n, rr) in enumerate(win_chunks[w]):
            for (i0, L, K, base) in rr:
                rmap_L[i0:i0 + L] = L
                rmap_K[i0:i0 + L] = K
                rmap_i0[i0:i0 + L] = i0
                rmap_gb[i0:i0 + L] = chunk_base[w][ci] + base
        m = sw == w
        rk = e_rank[m]
        s = slot[m]
        col = rmap_gb[rk] + (rk - rmap_i0[rk]) * rmap_K[rk] + s
        assert (rmap_gb[rk] >= 0).all()
        assert (s < rmap_K[rk]).all()
        cc = e_core[m]
        gg = e_group[m]
        vv = e_val[m]
        for c in range(C):
            mm = cc == c
            idx_tok[c][gg[mm], col[mm]] = vv[mm]

    idx_gather = [
        _wrap_groups(idx_tok[c]).astype(np.int16) for c in range(C)]

    # --- realign indices: per window, per group: canonical rank i -> window rank
    # gathered from acc_w [128, GSZ]
    align_tok = [np.zeros((C, G8, GSZ), dtype=np.int64) for _ in range(C)]
    for c in range(C):
        lo = c * NPC
        for g in range(G8):
            sel = np.flatnonzero(group_of[lo:lo + NPC] == g) + lo
            crk = rank_of[sel]
            for w in range(C):
                a = np.arange(GSZ, dtype=np.int64)
                a[crk] = wrank[sel, w]
                align_tok[c][w, g, :] = a
    idx_align = [
        np.concatenate([_wrap_groups(align_tok[c][w]) for w in range(C)],
                       axis=1).astype(np.int16) for c in range(C)]

    # --- pooling: per core, graphs touching its node range
    gfirst = np.searchsorted(batch, np.arange(N_GRAPHS))
    # graph of each node
    g_of_node = batch
    pool_runs = None
    pool_zero_from = None
    # counts per (core, slot, group); slots sorted by per-core touch-count desc
    core_graphs = []      # per core: list of graph ids in slot order
    slot_of = {}
    pcnt_max = np.zeros((PG,), dtype=np.int64)
    per_core_cnt = []
    for c in range(C):
        lo, hi = c * NPC, (c + 1) * NPC
        gids = np.unique(g_of_node[lo:hi])
        assert len(gids) <= PG - 1
        # count per (graph, group) among this core's nodes
        nodes = np.arange(lo, hi)
        k = (np.searchsorted(gids, g_of_node[lo:hi])) * G8 + group_of[lo:hi]
        cm = np.bincount(k, minlength=len(gids) * G8).reshape(len(gids), G8)
        order = np.argsort(-cm.sum(axis=1), kind="stable")
        gids_sorted = gids[order]
        cm = cm[order]
        core_graphs.append(gids_sorted)
        per_core_cnt.append(cm)
        m = cm.max(axis=1)
        pcnt_max[:len(gids)] = np.maximum(pcnt_max[:len(gids)], m)
    pool_runs, pool_zero_from = _build_runs(pcnt_max)
    pool_chunks = _chunk_runs(pool_runs, CHUNK)
    pool_base = []
    ptotal = 0
    for (n, _rr) in pool_chunks:
        pool_base.append(ptotal)
        ptotal += n
    PCOLS = ptotal // 16

    idx_pool = []
    for c in range(C):
        lo, hi = c * NPC, (c + 1) * NPC
        gids_sorted = core_graphs[c]
        slot_map = {gid: s for s, gid in enumerate(gids_sorted)}
        tok = np.full((G8, ptotal), GSZ, dtype=np.int64)   # GSZ = h zero col
        # rank->run lookup
        rmap_L = np.zeros(PG, dtype=np.int64)
        rmap_i0 = np.zeros(PG, dtype=np.int64)
        rmap_gb = np.full(PG, -1, dtype=np.int64)
        rmap_K = np.zeros(PG, dtype=np.int64)
        for ci, (n, rr) in enumerate(pool_chunks):
            for (i0, L, K, base) in rr:
                rmap_L[i0:i0 + L] = L
                rmap_K[i0:i0 + L] = K
                rmap_i0[i0:i0 + L] = i0
                rmap_gb[i0:i0 + L] = pool_base[ci] + base
        # per node of this core: slot, group, within count slot index
        nslots = np.array([slot_map[g] for g in g_of_node[lo:hi]])
        ngrp = group_of[lo:hi]
        nkey = nslots * G8 + ngrp
        order = np.argsort(nkey, kind="stable")
        ks = nkey[order]
        st = np.concatenate([[True], ks[1:] != ks[:-1]])
        fi = np.flatnonzero(st)
        gi = np.cumsum(st) - 1
        sl = np.arange(len(order)) - fi[gi]
        rk = nslots[order]
        col = rmap_gb[rk] + (rk - rmap_i0[rk]) * rmap_K[rk] + sl
        assert (rmap_gb[rk] >= 0).all() and (sl < rmap_K[rk]).all()
        tok[ngrp[order], col] = rank_of[lo:hi][order]
        idx_pool.append(_wrap_groups(tok).astype(np.int16))

    # --- assembly: per core, for its 125 output graphs: contributors
    idx_asm = []
    for c in range(C):
        tok = np.full((G8, PG), PZSLOT, dtype=np.int64)
        for j in range(GPC):
            gid = c * GPC + j
            for cc in range(C):
                pos = np.searchsorted(core_graphs[cc], gid)
                if pos < len(core_graphs[cc]) and core_graphs[cc][pos] == gid:
                    tok[cc, j] = pos
        idx_asm.append(_wrap_groups(tok).astype(np.int16))

    # --- per-core dense inputs
    cnt_graph = np.maximum(np.bincount(batch, minlength=N_GRAPHS), 1).astype(np.float32)
    per_core = []
    for c in range(C):
        lo = c * NPC
        xt = np.zeros((P, GSZ), dtype=np.float32)
        dis_cols = np.ones((P, GSZ), dtype=np.float32)
        nodes = np.arange(lo, lo + NPC)
        gg, rr_, = group_of[nodes], rank_of[nodes]
        for f in range(4):
            xt[gg * 16 + f, rr_] = x[nodes, f]
        for f in range(HID):
            dis_cols[gg * 16 + f, rr_] = dis[nodes]
        # xt is consumed by _make_inputs to build y1 on host
        rcp = np.ones((HID, PG), dtype=np.float32)
        rcp[:, :GPC] = 1.0 / cnt_graph[c * GPC:(c + 1) * GPC][None, :]
        per_core.append(dict(
            xt=xt, dis_cols=dis_cols,
            idx_gather=idx_gather[c], idx_align=idx_align[c],
            idx_pool=idx_pool[c], idx_asm=idx_asm[c], rcp=rcp))

    plan = dict(win_chunks=win_chunks, win_zero_from=win_zero_from,
                chunk_base=chunk_base, GCOLS=GCOLS,
                pool_chunks=pool_chunks, pool_zero_from=pool_zero_from,
                pool_base=pool_base, PCOLS=PCOLS)
    return per_core, plan


def _build_program(plan, reps=1, mode="full"):
    win_chunks = plan["win_chunks"]
    win_zero_from = plan["win_zero_from"]
    chunk_base = plan["chunk_base"]
    GCOLS = plan["GCOLS"]
    pool_chunks = plan["pool_chunks"]
    pool_zero_from = plan["pool_zero_from"]
    pool_base = plan["pool_base"]
    PCOLS = plan["PCOLS"]

    nc = bacc.Bacc(None, target_bir_lowering=False, num_devices=C,
                   num_swdge_queues=4)

    xt_in = nc.dram_tensor("xt", [P, GSZ], F32, kind="ExternalInput")
    dis_in = nc.dram_tensor("dis_cols", [P, GSZ], F32, kind="ExternalInput")
    ig_in = nc.dram_tensor("idx_gather", [P, GCOLS], I16, kind="ExternalInput")
    ia_in = nc.dram_tensor("idx_align", [P, C * GSZ // 16], I16, kind="ExternalInput")
    ip_in = nc.dram_tensor("idx_pool", [P, PCOLS], I16, kind="ExternalInput")
    im_in = nc.dram_tensor("idx_asm", [P, PG // 16], I16, kind="ExternalInput")
    rcp_in = nc.dram_tensor("rcp", [HID, PG], F32, kind="ExternalInput")
    kw_in = {}
    for l in range(1, 6):
        kw_in[l] = nc.dram_tensor(f"KW{l}", [P, P], F32, kind="ExternalInput")
    sel_in = nc.dram_tensor("sel", [P, HID], F32, kind="ExternalInput")
    bvec_in = nc.dram_tensor("bvec", [P, 8], F32, kind="ExternalInput")
    l1w_in = nc.dram_tensor("lin1_w", [HID, HID], F32, kind="ExternalInput")
    l2w_in = nc.dram_tensor("lin2_w", [HID, 1], F32, kind="ExternalInput")
    out_t = nc.dram_tensor("out", [1, PG], F32, kind="ExternalOutput")

    ag_in = nc.dram_tensor("ag_in", [HID, NPAD], F32)
    ag8 = nc.dram_tensor("ag8", [P, NPAD], F32)
    pool_out = nc.dram_tensor("pool_out", [HID, PG], F32)
    pool_ag = nc.dram_tensor("pool_ag", [P, PG], F32)

    core_id = nc.partition_id_tensor  # noqa: F841

    with tile.TileContext(nc) as tc:
        import contextlib
        with contextlib.ExitStack() as ctx:
            sbp = ctx.enter_context(tc.tile_pool(name="persist", bufs=1))
            gp = ctx.enter_context(tc.tile_pool(name="g", bufs=2))
            psp = ctx.enter_context(tc.tile_pool(name="ps", bufs=4, space="PSUM"))

            # persistent SBUF
            idx_g = sbp.tile([P, GCOLS], I16)
            nc.sync.dma_start(idx_g[:], ig_in[:])
            idx_a = sbp.tile([P, C * GSZ // 16], I16)
            nc.sync.dma_start(idx_a[:], ia_in[:])
            idx_p = sbp.tile([P, PCOLS], I16)
            nc.sync.dma_start(idx_p[:], ip_in[:])
            idx_m = sbp.tile([P, PG // 16], I16)
            nc.sync.dma_start(idx_m[:], im_in[:])

            xt_sb = sbp.tile([P, GSZ], F32)
            nc.sync.dma_start(xt_sb[:], xt_in[:])
            dis_sb = sbp.tile([P, GSZ], F32)
            nc.sync.dma_start(dis_sb[:], dis_in[:])
            rcp_sb = sbp.tile([HID, PG], F32)
            nc.sync.dma_start(rcp_sb[:], rcp_in[:])
            kw_sb = {}
            for l in range(1, 6):
                kw_sb[l] = sbp.tile([P, P], F32, tag=f"kw{l}", name=f"kw{l}")
                nc.sync.dma_start(kw_sb[l][:], kw_in[l][:])
            sel_sb = sbp.tile([P, HID], F32)
            nc.sync.dma_start(sel_sb[:], sel_in[:])
            bvec_sb = sbp.tile([P, 8], F32)
            nc.sync.dma_start(bvec_sb[:], bvec_in[:])
            l1w_sb = sbp.tile([HID, HID], F32)
            nc.sync.dma_start(l1w_sb[:], l1w_in[:])
            l2w_sb = sbp.tile([HID, 1], F32)
            nc.sync.dma_start(l2w_sb[:], l2w_in[:])

            y_own = sbp.tile([P, GSZ], F32)
            accw = [sbp.tile([P, GSZ], F32, tag=f"accw{i}", name=f"accw{i}")
                    for i in range(2)]
            pacc = sbp.tile([P, PG], F32)
            h_sb = sbp.tile([P, HSRC], F32)
            nc.vector.memset(h_sb[:, GSZ:], 0.0)
            win = [sbp.tile([P, WIN_ELEMS], F32, tag=f"win{i}", name=f"win{i}")
                   for i in range(2)]
            for i in range(2):
                nc.vector.memset(win[i][:, NPAD:], 0.0)

            def ap3(t, off, n):
                a = t[:]
                return bass.AP(a.tensor, a.offset + off,
                               [[a.ap[0][0], P], [1, n], [1, 1]])

            def gather(out_tile, out_off, src_tile, n_elems, idx_tile, idx_off, n):
                nc.gpsimd.ap_gather(
                    out_ap=ap3(out_tile, out_off, n),
                    in_ap=ap3(src_tile, 0, n_elems),
                    idxs_ap=idx_tile[:, idx_off:idx_off + n // 16],
                    channels=P, num_elems=n_elems, d=1, num_idxs=n)

            def reduce_chunk(g, runs, acc_w):
                for (i0, L, K, base) in runs:
                    a = g[:]
                    if K == 1:
                        nc.vector.tensor_copy(out=acc_w[:, i0:i0 + L],
                                              in_=g[:, base:base + L])
                        continue
                    in3 = bass.AP(a.tensor, a.offset + base,
                                  [[a.ap[0][0], P], [K, L], [1, K]])
                    nc.vector.reduce_sum(out=acc_w[:, i0:i0 + L], in_=in3,
                                         axis=mybir.AxisListType.X)

            def message_pass_abl(do_load=True, do_gather=True):
                for w in range(C):
                    wb = win[w % 2]
                    if do_load:
                        for g8 in range(G8):
                            src = bass.AP(ag8[:].tensor, (16 * w) * NPAD,
                                          [[NPAD, 16], [1, NPAD]])
                            nc.sync.dma_start(wb[16 * g8:16 * g8 + 16, :NPAD], src)
                    acc_w = accw[w % 2]
                    for ci, (n, runs) in enumerate(win_chunks[w]):
                        g = gp.tile([P, CHUNK + 32], F32, tag="g")
                        if do_gather:
                            gather(g, 0, wb, WIN_ELEMS, idx_g,
                                   chunk_base[w][ci] // 16, n)
                        else:
                            nc.vector.memset(g[:, :n], 0.5)
                        reduce_chunk(g, runs, acc_w)
                    zf = win_zero_from[w]
                    if zf < GSZ:
                        nc.vector.memset(acc_w[:, zf:], 0.0)
                    ga = gp.tile([P, GSZ], F32, tag="ga")
                    if do_gather:
                        gather(ga, 0, acc_w, GSZ, idx_a, w * GSZ // 16, GSZ)
                    else:
                        nc.vector.tensor_copy(out=ga[:], in_=acc_w[:])
                    nc.vector.tensor_add(out=y_own[:], in0=y_own[:], in1=ga[:])

            def message_pass(do_realign=True):
                """for each window: load, gather+reduce, realign, accumulate
                into y_own (which already holds the self contribution)."""
                for w in range(C):
                    wb = win[w % 2]
                    src = bass.AP(ag8[:].tensor, (16 * w) * NPAD,
                                  [[0, 8], [NPAD, 16], [1, NPAD]])
                    dst = bass.AP(wb[:].tensor, wb[:].offset,
                                  [[wb[:].ap[0][0], P], [1, NPAD]])
                    nc.sync.dma_start(dst, src)
                    acc_w = accw[w % 2]
                    for ci, (n, runs) in enumerate(win_chunks[w]):
                        g = gp.tile([P, CHUNK + 32], F32, tag="g")
                        gather(g, 0, wb, WIN_ELEMS, idx_g,
                               chunk_base[w][ci] // 16, n)
                        reduce_chunk(g, runs, acc_w)
                    zf = win_zero_from[w]
                    if zf < GSZ:
                        nc.vector.memset(acc_w[:, zf:], 0.0)
                    ga = gp.tile([P, GSZ], F32, tag="ga")
                    if do_realign:
                        gather(ga, 0, acc_w, GSZ, idx_a, w * GSZ // 16, GSZ)
                    else:
                        nc.vector.tensor_copy(out=ga[:], in_=acc_w[:])
                    nc.vector.tensor_add(out=y_own[:], in0=y_own[:], in1=ga[:])

            def epilogue(l):
                """y_own holds acc; compute h; if l<5 compute next y into
                y_own and write ag_in."""
                nc.vector.tensor_mul(out=y_own[:], in0=y_own[:], in1=dis_sb[:])
                nc.vector.tensor_scalar(out=h_sb[:, :GSZ], in0=y_own[:],
                                        scalar1=bvec_sb[:, l - 1:l],
                                        scalar2=0.0, op0=AL.add, op1=AL.max)
                if l < 5:
                    mm_from(h_sb, kw_sb[l + 1], write_ag=True)

            def mm_from(src_tile, w_tile, write_ag):
                """y_own = dis * (w_tile^T @ src_tile[:, :GSZ]); optionally
                write ag_in."""
                for b0 in range(0, GSZ, 512):
                    nb = min(512, GSZ - b0)
                    pm = psp.tile([P, 512], F32, tag="pm", space="PSUM")
                    nc.tensor.matmul(out=pm[:, :nb], lhsT=w_tile[:],
                                     rhs=src_tile[:, b0:b0 + nb],
                                     start=True, stop=True)
                    nc.vector.tensor_mul(out=y_own[:, b0:b0 + nb],
                                         in0=pm[:, :nb],
                                         in1=dis_sb[:, b0:b0 + nb])
                if write_ag:
                    for g8 in range(G8):
                        dst = bass.AP(ag_in[:].tensor, g8 * GSZ,
                                      [[NPAD, 16], [1, GSZ]])
                        nc.sync.dma_start(dst, y_own[16 * g8:16 * g8 + 16, :])

            def exchange():
                nc.gpsimd.collective_compute(
                    "AllGather", AL.bypass,
                    replica_groups=[list(range(C))],
                    ins=[ag_in[:]], outs=[ag8[:]])

            def layer1_y():
                # xt already holds y1 = dis * (x @ W1) (host precomputed)
                nc.vector.tensor_copy(out=y_own[:], in_=xt_sb[:])
                for g8 in range(G8):
                    dst = bass.AP(ag_in[:].tensor, g8 * GSZ,
                                  [[NPAD, 16], [1, GSZ]])
                    nc.sync.dma_start(dst, y_own[16 * g8:16 * g8 + 16, :])

            def pooling_and_head():
                for ci, (n, runs) in enumerate(pool_chunks):
                    g = gp.tile([P, CHUNK + 32], F32, tag="g")
                    gather(g, 0, h_sb, HSRC, idx_p, pool_base[ci] // 16, n)
                    reduce_chunk(g, runs, pacc)
                if pool_zero_from < PG:
                    nc.vector.memset(pacc[:, pool_zero_from:], 0.0)
                # cross-group sum -> [16, PG]
                pm = psp.tile([P, 512], F32, tag="pm", space="PSUM")
                nc.tensor.matmul(out=pm[:HID, :PG], lhsT=sel_sb[:], rhs=pacc[:],
                                 start=True, stop=True)
                psb = gp.tile([HID, PG], F32, tag="psb")
                nc.vector.tensor_copy(out=psb[:], in_=pm[:HID, :PG])
                nc.sync.dma_start(pool_out[:], psb[:])
                nc.gpsimd.collective_compute(
                    "AllGather", AL.bypass,
                    replica_groups=[list(range(C))],
                    ins=[pool_out[:]], outs=[pool_ag[:]])
                pag = gp.tile([P, PG], F32, tag="pag")
                nc.sync.dma_start(pag[:], pool_ag[:])
                asm = gp.tile([P, PG], F32, tag="asm")
                gather(asm, 0, pag, PG, idx_m, 0, PG)
                pt = psp.tile([P, 512], F32, tag="pm", space="PSUM")
                nc.tensor.matmul(out=pt[:HID, :PG], lhsT=sel_sb[:], rhs=asm[:],
                                 start=True, stop=True)
                tot = gp.tile([HID, PG], F32, tag="tot")
                nc.vector.tensor_mul(out=tot[:], in0=pt[:HID, :PG], in1=rcp_sb[:])

                def rrelu(dst_ap, src_ap, tmp_tag):
                    tmp = gp.tile([HID, PG], F32, tag=tmp_tag)
                    nc.vector.tensor_scalar(out=tmp[:src_ap.shape[0], :src_ap.shape[1]],
                                            in0=src_ap, scalar1=0.0,
                                            scalar2=None, op0=AL.max)
                    nc.vector.tensor_scalar(out=dst_ap, in0=src_ap, scalar1=0.0,
                                            scalar2=RRELU_SLOPE, op0=AL.min,
                                            op1=AL.mult)
                    nc.vector.tensor_add(out=dst_ap, in0=dst_ap,
                                         in1=tmp[:src_ap.shape[0], :src_ap.shape[1]])

                pm1 = psp.tile([P, 512], F32, tag="pm", space="PSUM")
                nc.tensor.matmul(out=pm1[:HID, :PG], lhsT=l1w_sb[:], rhs=tot[:],
                                 start=True, stop=True)
                g1 = gp.tile([HID, PG], F32, tag="g1")
                nc.vector.tensor_scalar(out=g1[:], in0=pm1[:HID, :PG],
                                        scalar1=bvec_sb[:HID, 5:6],
                                        scalar2=None, op0=AL.add)
                rrelu(g1[:], g1[:], "rr1")
                pm2 = psp.tile([P, 512], F32, tag="pm", space="PSUM")
                nc.tensor.matmul(out=pm2[:1, :PG], lhsT=l2w_sb[:], rhs=g1[:],
                                 start=True, stop=True)
                g2 = gp.tile([1, PG], F32, tag="g2")
                nc.vector.tensor_scalar(out=g2[:], in0=pm2[:1, :PG],
                                        scalar1=bvec_sb[:1, 6:7],
                                        scalar2=None, op0=AL.add)
                rrelu(g2[:], g2[:], "rr2")
                nc.sync.dma_start(out_t[:], g2[:])

            for _ in range(reps):
                if mode == "full":
                    layer1_y()
                    for l in range(1, 6):
                        exchange()
                        message_pass()
                        epilogue(l)
                    pooling_and_head()
                elif mode == "fullnoex":
                    layer1_y()
                    exchange()
                    for l in range(1, 6):
                        message_pass()
                        epilogue(l)
                    pooling_and_head()
                elif mode == "fullnogather":
                    layer1_y()
                    for l in range(1, 6):
                        exchange()
                        message_pass_abl(do_load=True, do_gather=False)
                        epilogue(l)
                    pooling_and_head()
                elif mode == "fullnoload":
                    layer1_y()
                    for l in range(1, 6):
                        exchange()
                        message_pass_abl(do_load=False, do_gather=True)
                        epilogue(l)
                    pooling_and_head()
                elif mode == "fullnorealign":
                    layer1_y()
                    for l in range(1, 6):
                        exchange()
                        message_pass(do_realign=False)
                        epilogue(l)
                    pooling_and_head()
                elif mode == "gathers":
                    layer1_y()
                    exchange()
                    for l in range(1, 6):
                        message_pass()
                    epilogue(5)
                    pooling_and_head()
                elif mode == "ag":
                    layer1_y()
                    for l in range(1, 6):
                        exchange()
                    epilogue(5)
                    pooling_and_head()
                elif mode == "agonly":
                    layer1_y()
                    for l in range(1, 6):
                        exchange()
                        nc.vector.tensor_copy(out=h_sb[:, :1],
                                              in_=y_own[:, :1])
                    nc.sync.dma_start(out_t[:], h_sb[:1, :PG])
                elif mode == "gonly":
                    layer1_y()
                    exchange()
                    for l in range(1, 6):
                        message_pass()
                    nc.sync.dma_start(out_t[:], y_own[:1, :PG])
                elif mode == "gnoload":
                    layer1_y()
                    exchange()
                    for w in range(C):
                        wb = win[w % 2]
                        for g8 in range(G8):
                            src = bass.AP(ag8[:].tensor, (16 * w) * NPAD,
                                          [[NPAD, 16], [1, NPAD]])
                            nc.sync.dma_start(wb[16 * g8:16 * g8 + 16, :NPAD], src)
                    for l in range(1, 6):
                        for w in range(C):
                            wb = win[w % 2]
                            acc_w = accw[w % 2]
                            for ci, (n, runs) in enumerate(win_chunks[w]):
                                g = gp.tile([P, CHUNK + 32], F32, tag="g")
                                gather(g, 0, wb, WIN_ELEMS, idx_g,
                                       chunk_base[w][ci] // 16, n)
                                reduce_chunk(g, runs, acc_w)
                            zf = win_zero_from[w]
                            if zf < GSZ:
                                nc.vector.memset(acc_w[:, zf:], 0.0)
                            ga = gp.tile([P, GSZ], F32, tag="ga")
                            gather(ga, 0, acc_w, GSZ, idx_a, w * GSZ // 16, GSZ)
                            nc.vector.tensor_add(out=y_own[:], in0=y_own[:], in1=ga[:])
                    nc.sync.dma_start(out_t[:], y_own[:1, :PG])
                elif mode == "gnored":
                    layer1_y()
                    exchange()
                    for l in range(1, 6):
                        for w in range(C):
                            wb = win[w % 2]
                            for ci, (n, runs) in enumerate(win_chunks[w]):
                                g = gp.tile([P, CHUNK + 32], F32, tag="g")
                                gather(g, 0, wb, WIN_ELEMS, idx_g,
                                       chunk_base[w][ci] // 16, n)
                    nc.sync.dma_start(out_t[:], y_own[:1, :PG])
                elif mode == "winonly":
                    layer1_y()
                    exchange()
                    for l in range(1, 6):
                        for w in range(C):
                            wb = win[w % 2]
                            for g8 in range(G8):
                                src = bass.AP(ag8[:].tensor, (16 * w) * NPAD,
                                              [[NPAD, 16], [1, NPAD]])
                                nc.sync.dma_start(wb[16 * g8:16 * g8 + 16, :NPAD], src)
                    nc.sync.dma_start(out_t[:], y_own[:1, :PG])
                elif mode == "winbig":
                    layer1_y()
                    exchange()
                    for l in range(1, 6):
                        for w in range(C):
                            wb = win[w % 2]
                            src = bass.AP(ag8[:].tensor, (16 * w) * NPAD,
                                          [[0, 8], [NPAD, 16], [1, NPAD]])
                            dst = bass.AP(wb[:].tensor, wb[:].offset,
                                          [[wb[:].ap[0][0], P], [1, NPAD]])
                            nc.sync.dma_start(dst, src)
                    nc.sync.dma_start(out_t[:], y_own[:1, :PG])
                elif mode == "winmulti":
                    layer1_y()
                    exchange()
                    engines = [nc.sync, nc.act, nc.vector, nc.sp]
                    for l in range(1, 6):
                        for w in range(C):
                            wb = win[w % 2]
                            for g8 in range(G8):
                                src = bass.AP(ag8[:].tensor, (16 * w) * NPAD,
                                              [[NPAD, 16], [1, NPAD]])
                                engines[g8 % 4].dma_start(
                                    wb[16 * g8:16 * g8 + 16, :NPAD], src)
                    nc.sync.dma_start(out_t[:], y_own[:1, :PG])
                elif mode == "epilogue":
                    layer1_y()
                    exchange()
                    message_pass()
                    for l in range(1, 6):
                        epilogue(min(l, 4))
                    pooling_and_head()

    nc.finalize()
    return nc


def _make_inputs(per_core, W1, W2, W3, W4, W5, b1, b2, b3, b4, b5,
                 lin1_w, lin1_b, lin2_w, lin2_b):
    Ws = [np.asarray(w, np.float32) for w in (W1, W2, W3, W4, W5)]
    bs = [np.asarray(b, np.float32) for b in (b1, b2, b3, b4, b5)]
    kws = []
    for i, W in enumerate(Ws):
        Wp = np.zeros((HID, HID), np.float32)
        Wp[:W.shape[0], :] = W
        kws.append(np.kron(np.eye(G8, dtype=np.float32), Wp))
    bvec = np.zeros((P, 8), np.float32)
    for l in range(5):
        for g in range(G8):
            bvec[16 * g:16 * g + 16, l] = bs[l]
    bvec[:HID, 5] = np.asarray(lin1_b, np.float32)
    bvec[0, 6] = np.asarray(lin2_b, np.float32).reshape(-1)[0]
    sel = np.zeros((P, HID), np.float32)
    for g in range(G8):
        sel[16 * g:16 * g + 16, :] = np.eye(HID, dtype=np.float32)

    in_maps = []
    for c in range(C):
        pc = per_core[c]
        y1 = (kws[0].T @ pc["xt"]) * pc["dis_cols"]
        m = {
            "xt": y1.astype(np.float32), "dis_cols": pc["dis_cols"],
            "idx_gather": pc["idx_gather"], "idx_align": pc["idx_align"],
            "idx_pool": pc["idx_pool"], "idx_asm": pc["idx_asm"],
            "rcp": pc["rcp"],
            "sel": sel, "bvec": bvec,
            "lin1_w": np.asarray(lin1_w, np.float32),
            "lin2_w": np.asarray(lin2_w, np.float32),
        }
        for l in range(1, 6):
            m[f"KW{l}"] = kws[l - 1]
        in_maps.append(m)
    return in_maps


def kernel(x, edge_index, batch, W1, b1, W2, b2, W3, b3, W4, b4, W5, b5,
           lin1_w, lin1_b, lin2_w, lin2_b, _reps=1, _prebuilt=None):
    per_core, plan = _preprocess(x, edge_index, batch)
    nc = _prebuilt if _prebuilt is not None else _build_program(plan, reps=_reps)
    in_maps = _make_inputs(per_core, W1, W2, W3, W4, W5, b1, b2, b3, b4, b5,
                           lin1_w, lin1_b, lin2_w, lin2_b)
    res = run_bass_via_pjrt(nc, in_maps, n_cores=C)
    out = np.zeros((N_GRAPHS, 1), dtype=np.float32)
    for c in range(C):
        out[c * GPC:(c + 1) * GPC, 0] = res[c]["out"][0, :GPC]
    return out


# revision 14
# speedup vs baseline: 12.4207x; 12.4207x over previous
"""GCN (5-layer) + global mean pool + MLP head on 8 trn2 NeuronCores.

Strategy (v2, feature-transposed + ap_gather):
  - Factorized GCN norm: y = dis * (h @ W); h'[v] = relu(dis[v]*(sum_in y + y[v]) + b).
  - Layout: features on partitions. Core c owns nodes [c*12500,(c+1)*12500),
    split into 8 groups of <=1568 dests; SBUF state tiles are [128, 1568]
    where partition 16g+f holds feature f of group g's dests.
  - Per layer, y is exchanged via AllGather into a LOCAL dram tensor
    ag8 [128, 12544] (row 16c+f = feature f of core c, col = canonical
    node column). Window w (= core w's slice) is DMA-replicated into SBUF
    [128, 12546] (all 8 groups see all of core w's nodes; 2 zero cols).
  - Message gathering runs on GPSIMD via ap_gather: each of the 8 Q7 cores
    gathers its group's in-edges' source columns. Columns are slot-major
    (k-major runs of constant K, dests degree-sorted per window); DVE
    tree-reduces runs and accumulates into acc; a realign ap_gather maps
    window-rank order back to canonical order.
  - Epilogue: h = relu(dis*acc + b); y' = dis * (kron(I8,W_next)^T @ h) via a
    single 128x128 PE matmul (block-diagonal weights keep group structure).
  - Pooling: partial per-core graph sums from own h5 (ap_gather + reduce),
    cross-group sum via a one-hot matmul, tiny AllGather of partials, then
    per-core assembly gather + MLP head.
"""
import numpy as np

import concourse.bass as bass
import concourse.bacc as bacc
import concourse.tile as tile
import concourse.mybir as mybir
from concourse.bass2jax import run_bass_via_pjrt

F32 = mybir.dt.float32
I16 = mybir.dt.int16
AL = mybir.AluOpType

N_NODES = 100000
N_EDGES = 3200000
N_GRAPHS = 1000
HID = 16
C = 8                    # cores
P = 128
NPC = N_NODES // C       # 12500
G8 = 8                   # partition groups (= gpsimd cores)
GSZ = 1568               # padded dests per group (8*1568 = 12544)
NPAD = G8 * GSZ          # 12544
WIN_ELEMS = NPAD + 2     # window cols incl 2 zero cols
ZCOL = NPAD              # zero column index in window
GPC = N_GRAPHS // C      # 125 graphs per core
PG = 128                 # pool slots per core (127 graphs max + zero slot)
PZSLOT = PG - 1          # zero slot in pool partials
HSRC = GSZ + 2           # h tile cols incl zero cols
CHUNK = 7168             # target idxs per ap_gather instruction
RRELU_SLOPE = (1.0 / 8.0 + 1.0 / 3.0) / 2.0


def _wrap_groups(idx_per_group):
    """[G8, n] int array -> [128, n//16] wrapped int16 (idx j of group g at
    partition 16g + j%16, col j//16)."""
    g8, n = idx_per_group.shape
    assert g8 == G8 and n % 16 == 0
    out = np.empty((P, n // 16), dtype=np.int16)
    for g in range(G8):
        out[16 * g:16 * g + 16, :] = idx_per_group[g].reshape(-1, 16).T
    return out


K_LEVELS = [1, 2, 3, 4, 6, 8, 12, 16, 24, 32, 48, 64, 96, 128]


def _quant_k(k):
    for lv in K_LEVELS:
        if k <= lv:
            return lv
    return int(k)


def _build_runs(Ks):
    """Monotone non-increasing K per rank -> list of (i0, L, K) runs, K>=1,
    with K quantized to K_LEVELS to merge runs."""
    runs = []
    i = 0
    n = len(Ks)
    while i < n and Ks[i] > 0:
        kq = _quant_k(int(Ks[i]))
        j = i
        while j < n and Ks[j] > 0 and _quant_k(int(Ks[j])) == kq:
            j += 1
        runs.append((i, j - i, kq))
        i = j
    return runs, i  # i = first zero-K rank


def _chunk_runs(runs, chunk_max):
    """Pack runs into chunks of <= chunk_max columns (padded to %32).
    Returns list of (n_idxs_padded, [(i0, L, K, base_col)])."""
    chunks = []
    cur = []
    cur_cols = 0
    for (i0, L, K) in runs:
        cols = L * K
        # oversize run: split by rank range
        while cols > chunk_max:
            take_L = max(1, chunk_max // K)
            if cur_cols + take_L * K > chunk_max and cur:
                chunks.append((cur_cols, cur))
                cur, cur_cols = [], 0
            take_L = min(take_L, L)
            cur.append((i0, take_L, K, cur_cols))
            cur_cols += take_L * K
            i0 += take_L
            L -= take_L
            cols = L * K
            if cur_cols >= chunk_max - 32:
                chunks.append((cur_cols, cur))
                cur, cur_cols = [], 0
        if cols == 0:
            continue
        if cur_cols + cols > chunk_max and cur:
            chunks.append((cur_cols, cur))
            cur, cur_cols = [], 0
        cur.append((i0, L, K, cur_cols))
        cur_cols += cols
    if cur:
        chunks.append((cur_cols, cur))
    out = []
    for (n, rr) in chunks:
        npad = (n + 31) // 32 * 32
        out.append((npad, rr))
    return out


def _preprocess(x, edge_index, batch):
    x = np.asarray(x, dtype=np.float32)
    src = np.asarray(edge_index[0], dtype=np.int64)
    dst = np.asarray(edge_index[1], dtype=np.int64)
    batch = np.asarray(batch, dtype=np.int64)

    deg = np.bincount(dst, minlength=N_NODES).astype(np.float32) + 1.0
    dis = 1.0 / np.sqrt(deg)

    # --- canonical positions: per core, sort by total degree desc, round-robin
    group_of = np.empty(N_NODES, dtype=np.int64)
    rank_of = np.empty(N_NODES, dtype=np.int64)
    for c in range(C):
        lo = c * NPC
        d = deg[lo:lo + NPC]
        order = np.argsort(-d, kind="stable")
        pos = np.empty(NPC, dtype=np.int64)
        pos[order] = np.arange(NPC)
        group_of[lo:lo + NPC] = pos % G8
        rank_of[lo:lo + NPC] = pos // G8
    col_of = group_of * GSZ + rank_of      # core-local canonical column
    wcol_of = col_of                       # window-local column (same)

    # --- per (dest, window) in-edge counts
    wsrc = src // NPC
    key_dw = dst * C + wsrc
    cnt = np.bincount(key_dw, minlength=N_NODES * C).reshape(N_NODES, C)

    # --- per (core, window, group): window ordering by count desc
    # wrank[d, w] = rank of dest d within (its core, its group) for window w
    wrank = np.empty((N_NODES, C), dtype=np.int64)
    Ks = np.zeros((C, GSZ), dtype=np.int64)   # [window, rank] cross core+group max
    for c in range(C):
        lo = c * NPC
        for g in range(G8):
            sel = np.flatnonzero(group_of[lo:lo + NPC] == g) + lo
            for w in range(C):
                cw = cnt[sel, w]
                order = np.argsort(-cw, kind="stable")
                rk = np.empty(len(sel), dtype=np.int64)
                rk[order] = np.arange(len(sel))
                wrank[sel, w] = rk
                Ks[w, :len(sel)] = np.maximum(Ks[w, :len(sel)], cw[order])

    # --- runs and chunks per window (shared across cores/groups)
    win_chunks = []   # per window: list of (n_idxs, [(i0,L,K,base)])
    win_zero_from = []
    for w in range(C):
        runs, zero_from = _build_runs(Ks[w])
        win_chunks.append(_chunk_runs(runs, CHUNK))
        win_zero_from.append(zero_from)

    # --- edge -> (sbuf col, partition) token assembly
    # chunk col offsets per window (sbuf idx tile layout: concat windows/chunks)
    chunk_base = []   # [w][ci] -> base idx-col (in idxs, not wrapped)
    total_idx = 0
    for w in range(C):
        bases = []
        for (n, _rr) in win_chunks[w]:
            bases.append(total_idx)
            total_idx += n
        chunk_base.append(bases)
    GCOLS = total_idx // 16

    # per-edge slot within (dest, window): stable order
    eorder = np.lexsort((src, wsrc, dst))
    sd, sw = dst[eorder], wsrc[eorder]
    gkey = sd * C + sw
    starts = np.concatenate([[True], gkey[1:] != gkey[:-1]])
    first = np.flatnonzero(starts)
    gidx = np.cumsum(starts) - 1
    slot = np.arange(len(eorder)) - first[gidx]

    # map (window, rank_in_window, slot) -> within-chunk column
    # build per window lookup arrays: for rank i: run id; run (i0, L, K, base, chunk_ci)
    # col_in_chunk = base + s*L + (i - i0); idx col global = chunk_base[w][ci] + col
    e_rank = wrank[sd, sw]
    e_group = group_of[sd]
    e_core = sd // NPC
    e_val = wcol_of[src[eorder]]

    idx_tok = [np.full((G8, total_idx), ZCOL, dtype=np.int64) for _ in range(C)]
    for w in range(C):
        # rank -> (L, K, i0, gbase) arrays
        rmap_L = np.zeros(GSZ, dtype=np.int64)
        rmap_K = np.zeros(GSZ, dtype=np.int64)
        rmap_i0 = np.zeros(GSZ, dtype=np.int64)
        rmap_gb = np.full(GSZ, -1, dtype=np.int64)
        for ci, (n, rr) in enumerate(win_chunks[w]):
            for (i0, L, K, base) in rr:
                rmap_L[i0:i0 + L] = L
                rmap_K[i0:i0 + L] = K
                rmap_i0[i0:i0 + L] = i0
                rmap_gb[i0:i0 + L] = chunk_base[w][ci] + base
        m = sw == w
        rk = e_rank[m]
        s = slot[m]
        col = rmap_gb[rk] + (rk - rmap_i0[rk]) * rmap_K[rk] + s
        assert (rmap_gb[rk] >= 0).all()
        assert (s < rmap_K[rk]).all()
        cc = e_core[m]
        gg = e_group[m]
        vv = e_val[m]
        for c in range(C):
            mm = cc == c
            idx_tok[c][gg[mm], col[mm]] = vv[mm]

    idx_gather = [
        _wrap_groups(idx_tok[c]).astype(np.int16) for c in range(C)]

    # --- realign indices: per window, per group: canonical rank i -> window rank
    # gathered from acc_w [128, GSZ]
    align_tok = [np.zeros((C, G8, GSZ), dtype=np.int64) for _ in range(C)]
    for c in range(C):
        lo = c * NPC
        for g in range(G8):
            sel = np.flatnonzero(group_of[lo:lo + NPC] == g) + lo
            crk = rank_of[sel]
            for w in range(C):
                a = np.arange(GSZ, dtype=np.int64)
                a[crk] = wrank[sel, w]
                align_tok[c][w, g, :] = a
    idx_align = [
        np.concatenate([_wrap_groups(align_tok[c][w]) for w in range(C)],
                       axis=1).astype(np.int16) for c in range(C)]

    # --- pooling: per core, graphs touching its node range
    gfirst = np.searchsorted(batch, np.arange(N_GRAPHS))
    # graph of each node
    g_of_node = batch
    pool_runs = None
    pool_zero_from = None
    # counts per (core, slot, group); slots sorted by per-core touch-count desc
    core_graphs = []      # per core: list of graph ids in slot order
    slot_of = {}
    pcnt_max = np.zeros((PG,), dtype=np.int64)
    per_core_cnt = []
    for c in range(C):
        lo, hi = c * NPC, (c + 1) * NPC
        gids = np.unique(g_of_node[lo:hi])
        assert len(gids) <= PG - 1
        # count per (graph, group) among this core's nodes
        nodes = np.arange(lo, hi)
        k = (np.searchsorted(gids, g_of_node[lo:hi])) * G8 + group_of[lo:hi]
        cm = np.bincount(k, minlength=len(gids) * G8).reshape(len(gids), G8)
        order = np.argsort(-cm.sum(axis=1), kind="stable")
        gids_sorted = gids[order]
        cm = cm[order]
        core_graphs.append(gids_sorted)
        per_core_cnt.append(cm)
        m = cm.max(axis=1)
        pcnt_max[:len(gids)] = np.maximum(pcnt_max[:len(gids)], m)
    pool_runs, pool_zero_from = _build_runs(pcnt_max)
    pool_chunks = _chunk_runs(pool_runs, CHUNK)
    pool_base = []
    ptotal = 0
    for (n, _rr) in pool_chunks:
        pool_base.append(ptotal)
        ptotal += n
    PCOLS = ptotal // 16

    idx_pool = []
    for c in range(C):
        lo, hi = c * NPC, (c + 1) * NPC
        gids_sorted = core_graphs[c]
        slot_map = {gid: s for s, gid in enumerate(gids_sorted)}
        tok = np.full((G8, ptotal), GSZ, dtype=np.int64)   # GSZ = h zero col
        # rank->run lookup
        rmap_L = np.zeros(PG, dtype=np.int64)
        rmap_i0 = np.zeros(PG, dtype=np.int64)
        rmap_gb = np.full(PG, -1, dtype=np.int64)
        rmap_K = np.zeros(PG, dtype=np.int64)
        for ci, (n, rr) in enumerate(pool_chunks):
            for (i0, L, K, base) in rr:
                rmap_L[i0:i0 + L] = L
                rmap_K[i0:i0 + L] = K
                rmap_i0[i0:i0 + L] = i0
                rmap_gb[i0:i0 + L] = pool_base[ci] + base
        # per node of this core: slot, group, within count slot index
        nslots = np.array([slot_map[g] for g in g_of_node[lo:hi]])
        ngrp = group_of[lo:hi]
        nkey = nslots * G8 + ngrp
        order = np.argsort(nkey, kind="stable")
        ks = nkey[order]
        st = np.concatenate([[True], ks[1:] != ks[:-1]])
        fi = np.flatnonzero(st)
        gi = np.cumsum(st) - 1
        sl = np.arange(len(order)) - fi[gi]
        rk = nslots[order]
        col = rmap_gb[rk] + (rk - rmap_i0[rk]) * rmap_K[rk] + sl
        assert (rmap_gb[rk] >= 0).all() and (sl < rmap_K[rk]).all()
        tok[ngrp[order], col] = rank_of[lo:hi][order]
        idx_pool.append(_wrap_groups(tok).astype(np.int16))

    # --- assembly: per core, for its 125 output graphs: contributors
    idx_asm = []
    for c in range(C):
        tok = np.full((G8, PG), PZSLOT, dtype=np.int64)
        for j in range(GPC):
            gid = c * GPC + j
            for cc in range(C):
                pos = np.searchsorted(core_graphs[cc], gid)
                if pos < len(core_graphs[cc]) and core_graphs[cc][pos] == gid:
                    tok[cc, j] = pos
        idx_asm.append(_wrap_groups(tok).astype(np.int16))

    # --- per-core dense inputs
    cnt_graph = np.maximum(np.bincount(batch, minlength=N_GRAPHS), 1).astype(np.float32)
    per_core = []
    for c in range(C):
        lo = c * NPC
        xt = np.zeros((P, GSZ), dtype=np.float32)
        dis_cols = np.ones((P, GSZ), dtype=np.float32)
        nodes = np.arange(lo, lo + NPC)
        gg, rr_, = group_of[nodes], rank_of[nodes]
        for f in range(4):
            xt[gg * 16 + f, rr_] = x[nodes, f]
        for f in range(HID):
            dis_cols[gg * 16 + f, rr_] = dis[nodes]
        # xt is consumed by _make_inputs to build y1 on host
        rcp = np.ones((HID, PG), dtype=np.float32)
        rcp[:, :GPC] = 1.0 / cnt_graph[c * GPC:(c + 1) * GPC][None, :]
        per_core.append(dict(
            xt=xt, dis_cols=dis_cols,
            idx_gather=idx_gather[c], idx_align=idx_align[c],
            idx_pool=idx_pool[c], idx_asm=idx_asm[c], rcp=rcp))

    plan = dict(win_chunks=win_chunks, win_zero_from=win_zero_from,
                chunk_base=chunk_base, GCOLS=GCOLS,
                pool_chunks=pool_chunks, pool_zero_from=pool_zero_from,
                pool_base=pool_base, PCOLS=PCOLS)
    return per_core, plan


def _build_program(plan, reps=1, mode="full"):
    win_chunks = plan["win_chunks"]
    win_zero_from = plan["win_zero_from"]
    chunk_base = plan["chunk_base"]
    GCOLS = plan["GCOLS"]
    pool_chunks = plan["pool_chunks"]
    pool_zero_from = plan["pool_zero_from"]
    pool_base = plan["pool_base"]
    PCOLS = plan["PCOLS"]

    nc = bacc.Bacc(None, target_bir_lowering=False, num_devices=C,
                   num_swdge_queues=4)

    xt_in = nc.dram_tensor("xt", [P, GSZ], F32, kind="ExternalInput")
    dis_in = nc.dram_tensor("dis_cols", [P, GSZ], F32, kind="ExternalInput")
    ig_in = nc.dram_tensor("idx_gather", [P, GCOLS], I16, kind="ExternalInput")
    ia_in = nc.dram_tensor("idx_align", [P, C * GSZ // 16], I16, kind="ExternalInput")
    ip_in = nc.dram_tensor("idx_pool", [P, PCOLS], I16, kind="ExternalInput")
    im_in = nc.dram_tensor("idx_asm", [P, PG // 16], I16, kind="ExternalInput")
    rcp_in = nc.dram_tensor("rcp", [HID, PG], F32, kind="ExternalInput")
    kw_in = {}
    for l in range(1, 6):
        kw_in[l] = nc.dram_tensor(f"KW{l}", [P, P], F32, kind="ExternalInput")
    sel_in = nc.dram_tensor("sel", [P, HID], F32, kind="ExternalInput")
    bvec_in = nc.dram_tensor("bvec", [P, 8], F32, kind="ExternalInput")
    l1w_in = nc.dram_tensor("lin1_w", [HID, HID], F32, kind="ExternalInput")
    l2w_in = nc.dram_tensor("lin2_w", [HID, 1], F32, kind="ExternalInput")
    out_t = nc.dram_tensor("out", [1, PG], F32, kind="ExternalOutput")

    ag_in = nc.dram_tensor("ag_in", [HID, NPAD], F32)
    ag8 = nc.dram_tensor("ag8", [P, NPAD], F32)
    pool_out = nc.dram_tensor("pool_out", [HID, PG], F32)
    pool_ag = nc.dram_tensor("pool_ag", [P, PG], F32)

    core_id = nc.partition_id_tensor  # noqa: F841

    with tile.TileContext(nc) as tc:
        import contextlib
        with contextlib.ExitStack() as ctx:
            sbp = ctx.enter_context(tc.tile_pool(name="persist", bufs=1))
            gp = ctx.enter_context(tc.tile_pool(name="g", bufs=2))
            psp = ctx.enter_context(tc.tile_pool(name="ps", bufs=1, space="PSUM"))

            # persistent SBUF
            idx_g = sbp.tile([P, GCOLS], I16)
            nc.sync.dma_start(idx_g[:], ig_in[:])
            idx_a = sbp.tile([P, C * GSZ // 16], I16)
            nc.sync.dma_start(idx_a[:], ia_in[:])
            idx_p = sbp.tile([P, PCOLS], I16)
            nc.sync.dma_start(idx_p[:], ip_in[:])
            idx_m = sbp.tile([P, PG // 16], I16)
            nc.sync.dma_start(idx_m[:], im_in[:])

            dis_sb = sbp.tile([P, GSZ], F32)
            nc.sync.dma_start(dis_sb[:], dis_in[:])
            rcp_sb = sbp.tile([HID, PG], F32)
            nc.sync.dma_start(rcp_sb[:], rcp_in[:])
            kw_sb = {}
            for l in range(1, 6):
                kw_sb[l] = sbp.tile([P, P], F32, tag=f"kw{l}", name=f"kw{l}")
                nc.sync.dma_start(kw_sb[l][:], kw_in[l][:])
            sel_sb = sbp.tile([P, HID], F32)
            nc.sync.dma_start(sel_sb[:], sel_in[:])
            bvec_sb = sbp.tile([P, 8], F32)
            nc.sync.dma_start(bvec_sb[:], bvec_in[:])
            l1w_sb = sbp.tile([HID, HID], F32)
            nc.sync.dma_start(l1w_sb[:], l1w_in[:])
            l2w_sb = sbp.tile([HID, 1], F32)
            nc.sync.dma_start(l2w_sb[:], l2w_in[:])

            y_own = sbp.tile([P, GSZ], F32)
            accw = [sbp.tile([P, GSZ], F32, tag="accw0", name="accw0")]
            pacc = sbp.tile([P, PG], F32)
            h_sb = sbp.tile([P, HSRC], F32)
            nc.vector.memset(h_sb[:, GSZ:], 0.0)
            win = [sbp.tile([P, WIN_ELEMS], F32, tag=f"win{i}", name=f"win{i}")
                   for i in range(2)]
            for i in range(2):
                nc.vector.memset(win[i][:, NPAD:], 0.0)

            def ap3(t, off, n):
                a = t[:]
                return bass.AP(a.tensor, a.offset + off,
                               [[a.ap[0][0], P], [1, n], [1, 1]])

            def gather(out_tile, out_off, src_tile, n_elems, idx_tile, idx_off, n):
                nc.gpsimd.ap_gather(
                    out_ap=ap3(out_tile, out_off, n),
                    in_ap=ap3(src_tile, 0, n_elems),
                    idxs_ap=idx_tile[:, idx_off:idx_off + n // 16],
                    channels=P, num_elems=n_elems, d=1, num_idxs=n)

            def reduce_chunk(g, runs, acc_w):
                for (i0, L, K, base) in runs:
                    a = g[:]
                    if K == 1:
                        nc.vector.tensor_copy(out=acc_w[:, i0:i0 + L],
                                              in_=g[:, base:base + L])
                        continue
                    in3 = bass.AP(a.tensor, a.offset + base,
                                  [[a.ap[0][0], P], [K, L], [1, K]])
                    nc.vector.reduce_sum(out=acc_w[:, i0:i0 + L], in_=in3,
                                         axis=mybir.AxisListType.X)

            def message_pass_abl(do_load=True, do_gather=True):
                for w in range(C):
                    wb = win[w % 2]
                    if do_load:
                        for g8 in range(G8):
                            src = bass.AP(ag8[:].tensor, (16 * w) * NPAD,
                                          [[NPAD, 16], [1, NPAD]])
                            nc.sync.dma_start(wb[16 * g8:16 * g8 + 16, :NPAD], src)
                    acc_w = accw[0]
                    for ci, (n, runs) in enumerate(win_chunks[w]):
                        g = gp.tile([P, CHUNK + 32], F32, tag="g")
                        if do_gather:
                            gather(g, 0, wb, WIN_ELEMS, idx_g,
                                   chunk_base[w][ci] // 16, n)
                        else:
                            nc.vector.memset(g[:, :n], 0.5)
                        reduce_chunk(g, runs, acc_w)
                    zf = win_zero_from[w]
                    if zf < GSZ:
                        nc.vector.memset(acc_w[:, zf:], 0.0)
                    ga = gp.tile([P, GSZ], F32, tag="ga")
                    if do_gather:
                        gather(ga, 0, acc_w, GSZ, idx_a, w * GSZ // 16, GSZ)
                    else:
                        nc.vector.tensor_copy(out=ga[:, :GSZ], in_=acc_w[:])
                    nc.vector.tensor_add(out=y_own[:], in0=y_own[:],
                                         in1=ga[:, :GSZ])

            def message_pass(do_realign=True):
                """for each window: load, gather+reduce, realign, accumulate
                into y_own (which already holds the self contribution)."""
                for w in range(C):
                    wb = win[w % 2]
                    src = bass.AP(ag8[:].tensor, (16 * w) * NPAD,
                                  [[0, 8], [NPAD, 16], [1, NPAD]])
                    dst = bass.AP(wb[:].tensor, wb[:].offset,
                                  [[wb[:].ap[0][0], P], [1, NPAD]])
                    nc.sync.dma_start(dst, src)
                    acc_w = accw[0]
                    for ci, (n, runs) in enumerate(win_chunks[w]):
                        g = gp.tile([P, CHUNK + 32], F32, tag="g")
                        gather(g, 0, wb, WIN_ELEMS, idx_g,
                               chunk_base[w][ci] // 16, n)
                        reduce_chunk(g, runs, acc_w)
                    zf = win_zero_from[w]
                    if zf < GSZ:
                        nc.vector.memset(acc_w[:, zf:], 0.0)
                    ga = gp.tile([P, CHUNK + 32], F32, tag="g")
                    if do_realign:
                        gather(ga, 0, acc_w, GSZ, idx_a, w * GSZ // 16, GSZ)
                    else:
                        nc.vector.tensor_copy(out=ga[:, :GSZ], in_=acc_w[:])
                    nc.vector.tensor_add(out=y_own[:], in0=y_own[:],
                                         in1=ga[:, :GSZ])

            def epilogue(l):
                """y_own holds acc; compute h; if l<5 compute next y into
                y_own and write ag_in."""
                nc.vector.tensor_mul(out=y_own[:], in0=y_own[:], in1=dis_sb[:])
                nc.vector.tensor_scalar(out=h_sb[:, :GSZ], in0=y_own[:],
                                        scalar1=bvec_sb[:, l - 1:l],
                                        scalar2=0.0, op0=AL.add, op1=AL.max)
                if l < 5:
                    mm_from(h_sb, kw_sb[l + 1], write_ag=True)

            def write_ag_in():
                dst = bass.AP(ag_in[:].tensor, 0,
                              [[GSZ, G8], [NPAD, 16], [1, GSZ]])
                src = bass.AP(y_own[:].tensor, y_own[:].offset,
                              [[y_own[:].ap[0][0], P], [1, GSZ]])
                nc.sync.dma_start(dst, src)

            def mm_from(src_tile, w_tile, write_ag):
                """y_own = dis * (w_tile^T @ src_tile[:, :GSZ]); optionally
                write ag_in."""
                pmw = psp.tile([P, 2048], F32, tag="pmw", space="PSUM")
                for b0 in range(0, GSZ, 512):
                    nb = min(512, GSZ - b0)
                    nc.tensor.matmul(out=pmw[:, b0:b0 + nb], lhsT=w_tile[:],
                                     rhs=src_tile[:, b0:b0 + nb],
                                     start=True, stop=True)
                nc.vector.tensor_mul(out=y_own[:], in0=pmw[:, :GSZ],
                                     in1=dis_sb[:])
                if write_ag:
                    write_ag_in()

            def exchange():
                nc.gpsimd.collective_compute(
                    "AllGather", AL.bypass,
                    replica_groups=[list(range(C))],
                    ins=[ag_in[:]], outs=[ag8[:]])

            def layer1_y():
                # xt already holds y1 = dis * (x @ W1) (host precomputed)
                nc.sync.dma_start(y_own[:], xt_in[:])
                write_ag_in()

            def pooling_and_head():
                for ci, (n, runs) in enumerate(pool_chunks):
                    g = gp.tile([P, CHUNK + 32], F32, tag="g")
                    gather(g, 0, h_sb, HSRC, idx_p, pool_base[ci] // 16, n)
                    reduce_chunk(g, runs, pacc)
                if pool_zero_from < PG:
                    nc.vector.memset(pacc[:, pool_zero_from:], 0.0)
                # cross-group sum -> [16, PG]
                pm = psp.tile([P, 512], F32, tag="pm", space="PSUM")
                nc.tensor.matmul(out=pm[:HID, :PG], lhsT=sel_sb[:], rhs=pacc[:],
                                 start=True, stop=True)
                psb = gp.tile([HID, PG], F32, tag="psb")
                nc.vector.tensor_copy(out=psb[:], in_=pm[:HID, :PG])
                nc.sync.dma_start(pool_out[:], psb[:])
                nc.gpsimd.collective_compute(
                    "AllGather", AL.bypass,
                    replica_groups=[list(range(C))],
                    ins=[pool_out[:]], outs=[pool_ag[:]])
                pag = gp.tile([P, PG], F32, tag="pag")
                nc.sync.dma_start(pag[:], pool_ag[:])
                asm = gp.tile([P, PG], F32, tag="asm")
                gather(asm, 0, pag, PG, idx_m, 0, PG)
                pt = psp.tile([P, 512], F32, tag="pm", space="PSUM")
                nc.tensor.matmul(out=pt[:HID, :PG], lhsT=sel_sb[:], rhs=asm[:],
                                 start=True, stop=True)
                tot = gp.tile([HID, PG], F32, tag="tot")
                nc.vector.tensor_mul(out=tot[:], in0=pt[:HID, :PG], in1=rcp_sb[:])

                def rrelu(dst_ap, src_ap, tmp_tag):
                    tmp = gp.tile([HID, PG], F32, tag=tmp_tag)
                    nc.vector.tensor_scalar(out=tmp[:src_ap.shape[0], :src_ap.shape[1]],
                                            in0=src_ap, scalar1=0.0,
                                            scalar2=None, op0=AL.max)
                    nc.vector.tensor_scalar(out=dst_ap, in0=src_ap, scalar1=0.0,
                                            scalar2=RRELU_SLOPE, op0=AL.min,
                                            op1=AL.mult)
                    nc.vector.tensor_add(out=dst_ap, in0=dst_ap,
                                         in1=tmp[:src_ap.shape[0], :src_ap.shape[1]])

                pm1 = psp.tile([P, 512], F32, tag="pm", space="PSUM")
                nc.tensor.matmul(out=pm1[:HID, :PG], lhsT=l1w_sb[:], rhs=tot[:],
                                 start=True, stop=True)
                g1 = gp.tile([HID, PG], F32, tag="g1")
                nc.vector.tensor_scalar(out=g1[:], in0=pm1[:HID, :PG],
                                        scalar1=bvec_sb[:HID, 5:6],
                                        scalar2=None, op0=AL.add)
                rrelu(g1[:], g1[:], "rr1")
                pm2 = psp.tile([P, 512], F32, tag="pm", space="PSUM")
                nc.tensor.matmul(out=pm2[:1, :PG], lhsT=l2w_sb[:], rhs=g1[:],
                                 start=True, stop=True)
                g2 = gp.tile([1, PG], F32, tag="g2")
                nc.vector.tensor_scalar(out=g2[:], in0=pm2[:1, :PG],
                                        scalar1=bvec_sb[:1, 6:7],
                                        scalar2=None, op0=AL.add)
                rrelu(g2[:], g2[:], "rr2")
                nc.sync.dma_start(out_t[:], g2[:])

            for _ in range(reps):
                if mode == "full":
                    layer1_y()
                    for l in range(1, 6):
                        exchange()
                        message_pass()
                        epilogue(l)
                    pooling_and_head()
                elif mode == "fullnoex":
                    layer1_y()
                    exchange()
                    for l in range(1, 6):
                        message_pass()
                        epilogue(l)
                    pooling_and_head()
                elif mode == "fullnogather":
                    layer1_y()
                    for l in range(1, 6):
                        exchange()
                        message_pass_abl(do_load=True, do_gather=False)
                        epilogue(l)
                    pooling_and_head()
                elif mode == "fullnoload":
                    layer1_y()
                    for l in range(1, 6):
                        exchange()
                        message_pass_abl(do_load=False, do_gather=True)
                        epilogue(l)
                    pooling_and_head()
                elif mode == "fullnorealign":
                    layer1_y()
                    for l in range(1, 6):
                        exchange()
                        message_pass(do_realign=False)
                        epilogue(l)
                    pooling_and_head()
                elif mode == "gathers":
                    layer1_y()
                    exchange()
                    for l in range(1, 6):
                        message_pass()
                    epilogue(5)
                    pooling_and_head()
                elif mode == "ag":
                    layer1_y()
                    for l in range(1, 6):
                        exchange()
                    epilogue(5)
                    pooling_and_head()
                elif mode == "agonly":
                    layer1_y()
                    for l in range(1, 6):
                        exchange()
                        nc.vector.tensor_copy(out=h_sb[:, :1],
                                              in_=y_own[:, :1])
                    nc.sync.dma_start(out_t[:], h_sb[:1, :PG])
                elif mode == "gonly":
                    layer1_y()
                    exchange()
                    for l in range(1, 6):
                        message_pass()
                    nc.sync.dma_start(out_t[:], y_own[:1, :PG])
                elif mode == "gnoload":
                    layer1_y()
                    exchange()
                    for w in range(C):
                        wb = win[w % 2]
                        for g8 in range(G8):
                            src = bass.AP(ag8[:].tensor, (16 * w) * NPAD,
                                          [[NPAD, 16], [1, NPAD]])
                            nc.sync.dma_start(wb[16 * g8:16 * g8 + 16, :NPAD], src)
                    for l in range(1, 6):
                        for w in range(C):
                            wb = win[w % 2]
                            acc_w = accw[0]
                            for ci, (n, runs) in enumerate(win_chunks[w]):
                                g = gp.tile([P, CHUNK + 32], F32, tag="g")
                                gather(g, 0, wb, WIN_ELEMS, idx_g,
                                       chunk_base[w][ci] // 16, n)
                                reduce_chunk(g, runs, acc_w)
                            zf = win_zero_from[w]
                            if zf < GSZ:
                                nc.vector.memset(acc_w[:, zf:], 0.0)
                            ga = gp.tile([P, GSZ], F32, tag="ga")
                            gather(ga, 0, acc_w, GSZ, idx_a, w * GSZ // 16, GSZ)
                            nc.vector.tensor_add(out=y_own[:], in0=y_own[:], in1=ga[:])
                    nc.sync.dma_start(out_t[:], y_own[:1, :PG])
                elif mode == "gnored":
                    layer1_y()
                    exchange()
                    for l in range(1, 6):
                        for w in range(C):
                            wb = win[w % 2]
                            for ci, (n, runs) in enumerate(win_chunks[w]):
                                g = gp.tile([P, CHUNK + 32], F32, tag="g")
                                gather(g, 0, wb, WIN_ELEMS, idx_g,
                                       chunk_base[w][ci] // 16, n)
                    nc.sync.dma_start(out_t[:], y_own[:1, :PG])
                elif mode == "winonly":
                    layer1_y()
                    exchange()
                    for l in range(1, 6):
                        for w in range(C):
                            wb = win[w % 2]
                            for g8 in range(G8):
                                src = bass.AP(ag8[:].tensor, (16 * w) * NPAD,
                                              [[NPAD, 16], [1, NPAD]])
                                nc.sync.dma_start(wb[16 * g8:16 * g8 + 16, :NPAD], src)
                    nc.sync.dma_start(out_t[:], y_own[:1, :PG])
                elif mode == "winbig":
                    layer1_y()
                    exchange()
                    for l in range(1, 6):
                        for w in range(C):
                            wb = win[w % 2]
                            src = bass.AP(ag8[:].tensor, (16 * w) * NPAD,
                                          [[0, 8], [NPAD, 16], [1, NPAD]])
                            dst = bass.AP(wb[:].tensor, wb[:].offset,
                                          [[wb[:].ap[0][0], P], [1, NPAD]])
                            nc.sync.dma_start(dst, src)
                    nc.sync.dma_start(out_t[:], y_own[:1, :PG])
                elif mode == "winmulti":
                    layer1_y()
                    exchange()
                    engines = [nc.sync, nc.act, nc.vector, nc.sp]
                    for l in range(1, 6):
                        for w in range(C):
                            wb = win[w % 2]
                            for g8 in range(G8):
                                src = bass.AP(ag8[:].tensor, (16 * w) * NPAD,
                                              [[NPAD, 16], [1, NPAD]])
                                engines[g8 % 4].dma_start(
                                    wb[16 * g8:16 * g8 + 16, :NPAD], src)
                    nc.sync.dma_start(out_t[:], y_own[:1, :PG])
                elif mode == "epilogue":
                    layer1_y()
                    exchange()
                    message_pass()
                    for l in range(1, 6):
                        epilogue(min(l, 4))
                    pooling_and_head()

    nc.finalize()
    return nc


def _make_inputs(per_core, W1, W2, W3, W4, W5, b1, b2, b3, b4, b5,
                 lin1_w, lin1_b, lin2_w, lin2_b):
    Ws = [np.asarray(w, np.float32) for w in (W1, W2, W3, W4, W5)]
    bs = [np.asarray(b, np.float32) for b in (b1, b2, b3, b4, b5)]
    kws = []
    for i, W in enumerate(Ws):
        Wp = np.zeros((HID, HID), np.float32)
        Wp[:W.shape[0], :] = W
        kws.append(np.kron(np.eye(G8, dtype=np.float32), Wp))
    bvec = np.zeros((P, 8), np.float32)
    for l in range(5):
        for g in range(G8):
            bvec[16 * g:16 * g + 16, l] = bs[l]
    bvec[:HID, 5] = np.asarray(lin1_b, np.float32)
    bvec[0, 6] = np.asarray(lin2_b, np.float32).reshape(-1)[0]
    sel = np.zeros((P, HID), np.float32)
    for g in range(G8):
        sel[16 * g:16 * g + 16, :] = np.eye(HID, dtype=np.float32)

    in_maps = []
    for c in range(C):
        pc = per_core[c]
        y1 = (kws[0].T @ pc["xt"]) * pc["dis_cols"]
        m = {
            "xt": y1.astype(np.float32), "dis_cols": pc["dis_cols"],
            "idx_gather": pc["idx_gather"], "idx_align": pc["idx_align"],
            "idx_pool": pc["idx_pool"], "idx_asm": pc["idx_asm"],
            "rcp": pc["rcp"],
            "sel": sel, "bvec": bvec,
            "lin1_w": np.asarray(lin1_w, np.float32),
            "lin2_w": np.asarray(lin2_w, np.float32),
        }
        for l in range(1, 6):
            m[f"KW{l}"] = kws[l - 1]
        in_maps.append(m)
    return in_maps


def kernel(x, edge_index, batch, W1, b1, W2, b2, W3, b3, W4, b4, W5, b5,
           lin1_w, lin1_b, lin2_w, lin2_b, _reps=1, _prebuilt=None):
    per_core, plan = _preprocess(x, edge_index, batch)
    nc = _prebuilt if _prebuilt is not None else _build_program(plan, reps=_reps)
    in_maps = _make_inputs(per_core, W1, W2, W3, W4, W5, b1, b2, b3, b4, b5,
                           lin1_w, lin1_b, lin2_w, lin2_b)
    res = run_bass_via_pjrt(nc, in_maps, n_cores=C)
    out = np.zeros((N_GRAPHS, 1), dtype=np.float32)
    for c in range(C):
        out[c * GPC:(c + 1) * GPC, 0] = res[c]["out"][0, :GPC]
    return out
